# revision 12
# baseline (speedup 1.0000x reference)
"""AgriMatcher Trainium2 kernel.

Strategy (8 NeuronCores, B=2 batches, N=4096, C=128):
  - Data-parallel over batch: cores 0-3 handle batch 0, cores 4-7 batch 1.
  - Within a batch the [N,N] matrix is row-sharded 4 ways (1024 rows/core).
  - Sinkhorn runs in probability domain: K = exp(S/tau), a = exp(u),
    b = exp(v).  Row pass s = K b is local (b replicated); col pass
    t = K^T a produces partials AllReduce'd over the 4-core group, split
    into two halves so communication overlaps compute.  The dustbin
    row/column are handled algebraically with replicated scalars.
  - K is stored twice in bf16 (row-major for the col pass via PE matvec,
    transposed - produced by DMA-transpose - for the row pass).
  - The final assignment matrix is recomputed in fp32:
    t_ij = exp(cos_ij/tau + ln b_j) via a rank-1 PSUM accumulate, so top-8
    selection and values match the fp32 reference closely.  Top-8 via DVE
    max8 + find_index8.
  - Host does the small tail (geo validation, softmax, affine solve) in
    float64 on [B,N,8] data.
"""

import math
import numpy as np

B = 2
N = 4096
C = 128
P = 128
NT_A = 8     # row tiles per core (1024 rows)
NT_B = 32    # column tiles (full N)
ROWS = NT_A * P
H = W = 64
TOPK = 8
ITERS = 5

_CACHE = {}


def _build():
    import os as _os
    import concourse.bass as bass
    from concourse import bacc, masks
    import concourse.mybir as mybir
    from concourse import tile

    dt = mybir.dt
    AF = mybir.ActivationFunctionType
    DBG = _os.environ.get("AGRI_DEBUG", "0") == "1"

    nc = bacc.Bacc("TRN2", target_bir_lowering=False, debug=False, num_devices=8)

    fa = nc.dram_tensor("fa", [ROWS, C], dt.float32, kind="ExternalInput")
    fb = nc.dram_tensor("fb", [N, C], dt.float32, kind="ExternalInput")
    # scal: [inv_temp, kd, mu, muD, nuD, tau, 0, 0]
    scal = nc.dram_tensor("scal", [1, 8], dt.float32, kind="ExternalInput")
    vals_out = nc.dram_tensor("vals", [ROWS, TOPK], dt.float32, kind="ExternalOutput")
    idx_out = nc.dram_tensor("idx", [ROWS, TOPK], dt.uint32, kind="ExternalOutput")
    if DBG:
        dbg_a5 = nc.dram_tensor("dbg_a5", [P, NT_A], dt.float32, kind="ExternalOutput")
        dbg_b5 = nc.dram_tensor("dbg_b5", [P, NT_B], dt.float32, kind="ExternalOutput")

    with tile.TileContext(nc) as tc:
        with tc.tile_pool(name="persist", bufs=1) as pp, \
             tc.tile_pool(name="dram", bufs=2, space="DRAM") as dram, \
             tc.tile_pool(name="psum", bufs=1, space="PSUM") as psq:
            faT = pp.tile([P, ROWS], dt.float32)         # [c, i] normalized A
            fbT = pp.tile([P, N], dt.float32)            # [c, j] normalized B
            scal_rep = pp.tile([P, 8], dt.float32)
            ones128 = pp.tile([P, 1], dt.float32)
            ones1 = pp.tile([1, P], dt.float32)
            ident = pp.tile([P, P], dt.float32)
            rs_2d = pp.tile([P, NT_A], dt.float32)       # phase-1 row sums
            rs_acc = pp.tile([P, 4], dt.float32)         # per-quarter accums
            a_2d = pp.tile([P, NT_A], dt.float32)
            b_2d = pp.tile([P, NT_B], dt.float32)
            a_bf = pp.tile([P, NT_A], dt.bfloat16)
            b_bf = pp.tile([P, NT_B], dt.bfloat16)
            aD_rep = pp.tile([P, 1], dt.float32)
            bD_rep = pp.tile([P, 1], dt.float32)
            tmp_rep = pp.tile([P, 1], dt.float32)
            tmp_rep2 = pp.tile([P, 1], dt.float32)
            kdbD = pp.tile([P, 1], dt.float32)
            sc1 = pp.tile([1, 1], dt.float32)

            nc.vector.memset(ones128[:], 1.0)
            nc.vector.memset(ones1[:], 1.0)
            masks.make_identity(nc, ident[:])

            INV_T, KD, MU, MUD, NUD, TAU = 0, 1, 2, 3, 4, 5

            def rep_col(k):
                return scal_rep[:, k:k + 1]

            def rep1(dst, src_11):
                """replicate a [1,1] sbuf value to [128,1] via PE rank-1"""
                psr = psq.tile([P, 1], dt.float32, tag="tiny")
                nc.tensor.matmul(psr[:], ones1[:], src_11, start=True, stop=True)
                nc.vector.tensor_copy(dst, psr[:])

            def part_sum(dst_11, src_p1):
                """sum a [128,1] over partitions -> [1,1] sbuf"""
                pss = psq.tile([1, 1], dt.float32, tag="tiny2")
                nc.tensor.matmul(pss[:], ones128[:], src_p1, start=True, stop=True)
                nc.vector.tensor_copy(dst_11, pss[:])

            # ---------------- phase 0: load, normalize, transpose ----------
            with tc.tile_pool(name="ph0", bufs=1) as p0:
                scal_sb = p0.tile([1, 8], dt.float32)
                nc.sync.dma_start(scal_sb[:], scal[:])
                ps_scal = psq.tile([P, 8], dt.float32, tag="tiny")
                nc.tensor.matmul(ps_scal[:], ones1[:], scal_sb[:], start=True, stop=True)
                nc.vector.tensor_copy(scal_rep[:], ps_scal[:])

                fa_buf = p0.tile([P, NT_A, C], dt.float32)
                fb_buf = p0.tile([P, NT_B, C], dt.float32)
                nc.sync.dma_start(fa_buf[:], fa.ap().rearrange("(t p) c -> p t c", p=P))
                nc.sync.dma_start(fb_buf[:], fb.ap().rearrange("(t p) c -> p t c", p=P))

                for name, buf, nt, dst in (("a", fa_buf, NT_A, faT), ("b", fb_buf, NT_B, fbT)):
                    sq = p0.tile([P, nt, C], dt.float32, tag=f"sq{name}")
                    ssq = p0.tile([P, nt], dt.float32, tag=f"ssq{name}")
                    nc.scalar.activation(sq[:], buf[:], AF.Square)
                    nc.vector.tensor_reduce(ssq[:], sq[:], axis=mybir.AxisListType.X,
                                            op=mybir.AluOpType.add)
                    rinv = p0.tile([P, nt], dt.float32, tag=f"rinv{name}")
                    nc.vector.reciprocal(rinv[:], ssq[:])
                    nc.scalar.activation(rinv[:], rinv[:], AF.Sqrt)  # 1/|row|
                    nc.vector.tensor_mul(
                        buf[:], buf[:],
                        rinv[:].unsqueeze(2).broadcast_to([P, nt, C]))
                    # transpose tiles via PE into [c, n] blocks
                    for blk in range((nt + 3) // 4):
                        ntile = min(4, nt - blk * 4)
                        pst = psq.tile([P, 4 * P], dt.float32, tag="q")
                        for q in range(ntile):
                            t_i = blk * 4 + q
                            nc.tensor.transpose(pst[:, q * P:(q + 1) * P],
                                                buf[:, t_i, :], ident[:])
                        nc.vector.tensor_copy(
                            dst[:, blk * 4 * P: blk * 4 * P + ntile * P],
                            pst[:, 0:ntile * P])

            # ---------------- phase 1: S matmul + exp -> K (bf16) ----------
            kp_cm = tc.tile_pool(name="kpool", bufs=1)
            kp = kp_cm.__enter__()
            K_rm = kp.tile([P, NT_A, N], dt.bfloat16)    # row-major K
            KT = kp.tile([P, NT_B, ROWS], dt.bfloat16)   # transposed K
            itv_cm = tc.tile_pool(name="itv", bufs=1)
            itv = itv_cm.__enter__()
            s_row = itv.tile([1, ROWS], dt.float32)
            t_rowh = itv.tile([1, 2048], dt.float32)
            for it in range(NT_A):
                for qq in range(4):
                    ps_s = psq.tile([P, 1024], dt.float32, tag="q")
                    for h in range(2):
                        j0 = qq * 1024 + h * 512
                        nc.tensor.matmul(ps_s[:, h * 512:(h + 1) * 512],
                                         faT[:, it * P:(it + 1) * P],
                                         fbT[:, j0:j0 + 512],
                                         start=True, stop=True)
                    nc.scalar.activation(K_rm[:, it, qq * 1024:(qq + 1) * 1024],
                                         ps_s[:], AF.Exp,
                                         scale=rep_col(INV_T),
                                         accum_out=rs_acc[:, qq:qq + 1])
                nc.vector.tensor_add(rs_2d[:, it:it + 1],
                                     rs_acc[:, 0:1], rs_acc[:, 1:2])
                nc.vector.tensor_add(tmp_rep[:], rs_acc[:, 2:3], rs_acc[:, 3:4])
                nc.vector.tensor_add(rs_2d[:, it:it + 1],
                                     rs_2d[:, it:it + 1], tmp_rep[:])
                # transposed copy on the DMA engine
                nc.sync.dma_start_transpose(
                    KT[:, :, it * P:(it + 1) * P], K_rm[:, it, :])

            # ---------------- Sinkhorn iterations --------------------------
            for itr in range(ITERS):
                # ---- a-update: s_i = sum_j K_ij b_j + kd*bD ----
                if itr == 0:
                    # b = 1: s = rowsums + kd ; aD = muD/(kd*N + 1)
                    nc.vector.tensor_scalar_add(a_2d[:], rs_2d[:], rep_col(KD))
                    nc.vector.reciprocal(a_2d[:], a_2d[:])
                    nc.vector.tensor_scalar_mul(a_2d[:], a_2d[:], rep_col(MU))
                    nc.vector.tensor_scalar_mul(aD_rep[:], ones128[:], rep_col(KD))
                    nc.vector.tensor_scalar(aD_rep[:], aD_rep[:], float(N), 1.0,
                                            op0=mybir.AluOpType.mult,
                                            op1=mybir.AluOpType.add)
                    nc.vector.reciprocal(aD_rep[:], aD_rep[:])
                    nc.vector.tensor_scalar_mul(aD_rep[:], aD_rep[:], rep_col(MUD))
                else:
                    ps_row = psq.tile([1, ROWS], dt.float32, tag="rowvec")
                    for g in range(NT_B):
                        nc.tensor.matmul(ps_row[:, 0:512], b_bf[:, g:g + 1],
                                         KT[:, g, 0:512],
                                         start=(g == 0), stop=False)
                        nc.tensor.matmul(ps_row[:, 512:1024], b_bf[:, g:g + 1],
                                         KT[:, g, 512:1024],
                                         start=(g == 0), stop=(g == NT_B - 1))
                    nc.scalar.copy(s_row[:], ps_row[:])
                    s_dram = dram.tile([ROWS], dt.float32, tag="s_dram")
                    nc.sync.dma_start(s_dram[:], s_row[:])
                    s_2d = a_2d  # reuse storage
                    nc.sync.dma_start(
                        s_2d[:], s_dram[:].rearrange("(t p) -> p t", p=P))
                    nc.vector.tensor_scalar_add(s_2d[:], s_2d[:], kdbD[:])
                    nc.vector.reciprocal(s_2d[:], s_2d[:])
                    nc.vector.tensor_scalar_mul(a_2d[:], s_2d[:], rep_col(MU))
                    # aD = muD / (kd*sum_b + bD)
                    nc.vector.tensor_reduce(tmp_rep[:], b_2d[:],
                                            axis=mybir.AxisListType.X,
                                            op=mybir.AluOpType.add)
                    part_sum(sc1[:], tmp_rep[:])
                    rep1(tmp_rep[:], sc1[:])
                    nc.vector.tensor_mul(tmp_rep[:], tmp_rep[:], rep_col(KD))
                    nc.vector.tensor_add(tmp_rep[:], tmp_rep[:], bD_rep[:])
                    nc.vector.reciprocal(tmp_rep[:], tmp_rep[:])
                    nc.vector.tensor_mul(aD_rep[:], tmp_rep[:], rep_col(MUD))
                if DBG and itr == ITERS - 1:
                    nc.sync.dma_start(dbg_a5.ap(), a_2d[:])
                nc.vector.tensor_copy(a_bf[:], a_2d[:])
                # local sum of a for the dustbin column term
                nc.vector.tensor_reduce(tmp_rep2[:], a_2d[:],
                                        axis=mybir.AxisListType.X,
                                        op=mybir.AluOpType.add)
                part_sum(sc1[:], tmp_rep2[:])

                # ---- col pass: two AR halves, each two psum quarters ----
                ar_halves = []
                for hf in range(2):
                    ar_in = dram.tile([2056], dt.float32, tag=f"ar_in{hf}")
                    ar_out = dram.tile([2056], dt.float32, tag=f"ar_out{hf}")
                    for qq in range(2):
                        ps_col = psq.tile([1, 1024], dt.float32, tag="q")
                        for it in range(NT_A):
                            for h in range(2):
                                j0 = hf * 2048 + qq * 1024 + h * 512
                                nc.tensor.matmul(ps_col[:, h * 512:(h + 1) * 512],
                                                 a_bf[:, it:it + 1],
                                                 K_rm[:, it, j0:j0 + 512],
                                                 start=(it == 0),
                                                 stop=(it == NT_A - 1))
                        nc.scalar.copy(t_rowh[:, qq * 1024:(qq + 1) * 1024], ps_col[:])
                    nc.sync.dma_start(ar_in[0:2048], t_rowh[:, 0:2048])
                    if hf == 0:
                        nc.sync.dma_start(ar_in[2048:2049], sc1[:])
                    nc.gpsimd.collective_compute(
                        "AllReduce", mybir.AluOpType.add,
                        replica_groups=[[0, 1, 2, 3], [4, 5, 6, 7]],
                        ins=[ar_in.opt()], outs=[ar_out.opt()])
                    ar_halves.append(ar_out)

                # ---- b-update (dustbin pieces from half 0's extra slot) ----
                suma_sb = sc1
                nc.sync.dma_start(suma_sb[:], ar_halves[0][2048:2049])
                rep1(tmp_rep[:], suma_sb[:])
                nc.vector.tensor_mul(tmp_rep[:], tmp_rep[:], rep_col(KD))
                nc.vector.tensor_add(tmp_rep[:], tmp_rep[:], aD_rep[:])
                nc.vector.reciprocal(tmp_rep[:], tmp_rep[:])
                nc.vector.tensor_mul(bD_rep[:], tmp_rep[:], rep_col(NUD))
                nc.vector.tensor_mul(kdbD[:], bD_rep[:], rep_col(KD))
                nc.vector.tensor_mul(tmp_rep2[:], aD_rep[:], rep_col(KD))
                for hf in range(2):
                    bh = b_2d[:, hf * 16:(hf + 1) * 16]
                    nc.sync.dma_start(
                        bh, ar_halves[hf][0:2048].rearrange("(g p) -> p g", p=P))
                    nc.vector.tensor_scalar_add(bh, bh, tmp_rep2[:])
                    nc.vector.reciprocal(bh, bh)
                    nc.vector.tensor_scalar_mul(bh, bh, rep_col(MU))
                    nc.vector.tensor_copy(b_bf[:, hf * 16:(hf + 1) * 16], bh)
                if DBG and itr == ITERS - 1:
                    nc.sync.dma_start(dbg_b5.ap(), b_2d[:])

            # ---------------- final: fp32 rescore + top-8 -------------------
            itv_cm.__exit__(None, None, None)
            with tc.tile_pool(name="fin", bufs=2) as fp, \
                 tc.tile_pool(name="fl", bufs=1) as fl, \
                 tc.tile_pool(name="finsmall", bufs=3) as fs:
                lnb_row = fl.tile([1, N], dt.float32)
                lnb_2d = fs.tile([P, NT_B], dt.float32, tag="lnb2d")
                nc.scalar.activation(lnb_2d[:], b_2d[:], AF.Ln)
                nc.vector.tensor_scalar_mul(lnb_2d[:], lnb_2d[:], rep_col(TAU))
                lnb_dram = dram.tile([N], dt.float32, tag="lnb_dram")
                nc.sync.dma_start(
                    lnb_dram[:].rearrange("(g p) -> p g", p=P), lnb_2d[:])
                nc.sync.dma_start(lnb_row[:], lnb_dram[:])

                for it in range(NT_A):
                    tsc = fp.tile([P, N], dt.float32, tag="tscratch")
                    for qq in range(4):
                        ps_t = psq.tile([P, 1024], dt.float32, tag="q")
                        for h in range(2):
                            j0 = qq * 1024 + h * 512
                            nc.tensor.matmul(ps_t[:, h * 512:(h + 1) * 512],
                                             faT[:, it * P:(it + 1) * P],
                                             fbT[:, j0:j0 + 512],
                                             start=True, stop=False)
                        for h in range(2):
                            j0 = qq * 1024 + h * 512
                            nc.tensor.matmul(ps_t[:, h * 512:(h + 1) * 512],
                                             ones1[:],
                                             lnb_row[0:1, j0:j0 + 512],
                                             start=False, stop=True)
                        nc.scalar.activation(tsc[:, qq * 1024:(qq + 1) * 1024],
                                             ps_t[:], AF.Exp, scale=rep_col(INV_T))
                    v8 = fs.tile([P, TOPK], dt.float32, tag="v8")
                    i8 = fs.tile([P, TOPK], dt.uint32, tag="i8")
                    nc.vector.max(v8[:], tsc[:])
                    nc.vector.max_index(i8[:], v8[:], tsc[:])
                    nc.vector.tensor_scalar_mul(v8[:], v8[:], a_2d[:, it:it + 1])
                    nc.sync.dma_start(
                        vals_out.ap().rearrange("(t p) k -> p t k", p=P)[:, it, :],
                        v8[:])
                    nc.sync.dma_start(
                        idx_out.ap().rearrange("(t p) k -> p t k", p=P)[:, it, :],
                        i8[:])

            kp_cm.__exit__(None, None, None)

    nc.compile()
    return nc


def _get_nc():
    if "nc" not in _CACHE:
        _CACHE["nc"] = _build()
    return _CACHE["nc"]


def _host_tail(vals, idx, pos_A, pos_B, gw, temp):
    """vals/idx: [B, N, 8];  tail in float64 on host."""
    import numpy as _np
    Bq, Nq, K = vals.shape
    vals = vals.astype(_np.float64)
    pa = pos_A.astype(_np.float64)
    pb = pos_B.astype(_np.float64)
    gw = float(_np.clip(gw, 0.0, 2.0))
    bidx = _np.arange(Bq)[:, None, None]
    tpos = pb[bidx, idx]                      # [B,N,K,2]
    disp = (tpos - pa[:, :, None, :]).reshape(Bq, H, W, K, 2)

    def box(x):
        # 7x7 zero-padded box sum via cumsum
        for ax in (1, 2):
            cs = _np.cumsum(x, axis=ax)
            n = x.shape[ax]
            hi = _np.clip(_np.arange(n) + 3, 0, n - 1)
            lo = _np.arange(n) - 4
            hi_s = _np.take(cs, hi, axis=ax)
            lo_s = _np.take(cs, _np.clip(lo, 0, n - 1), axis=ax)
            mask_shape = (1,) * ax + (n,) + (1,) * (x.ndim - ax - 1)
            lo_s = _np.where((lo >= 0).reshape(mask_shape), lo_s, 0.0)
            x = hi_s - lo_s
        return x

    cnt = box(_np.ones((1, H, W, 1, 1)))
    mean = box(disp) / cnt
    mean_sq = box(disp * disp) / cnt
    var = _np.clip(mean_sq - mean * mean, 0.0, None)
    geo = 1.0 / (1.0 + 100.0 * var.sum(-1).reshape(Bq, Nq, K))

    comb = vals + gw * geo
    e = _np.exp((comb - comb.max(-1, keepdims=True)) / temp)
    soft = e / e.sum(-1, keepdims=True)
    warp = (tpos * soft[..., None]).sum(2)
    conf = (soft * vals).sum(-1, keepdims=True)

    w = conf[..., 0]
    w = w / _np.clip(w.sum(1, keepdims=True), 1e-6, None)
    x, y = pa[..., 0], pa[..., 1]
    one = _np.ones_like(x)
    zero = _np.zeros_like(x)
    rx = _np.stack([x, y, one, zero, zero, zero], -1)
    ry = _np.stack([zero, zero, zero, x, y, one], -1)
    A = _np.concatenate([rx, ry], 1)
    bb = _np.concatenate([warp[..., 0], warp[..., 1]], 1)[..., None]
    wd = _np.concatenate([w, w], 1)[..., None]
    Aw = A * wd
    AtWA = _np.einsum('bni,bnj->bij', Aw, A) + 1e-4 * _np.eye(6)
    AtWb = _np.einsum('bni,bnk->bik', Aw, bb)
    p = _np.linalg.solve(AtWA, AtWb)[..., 0]
    z = _np.zeros(Bq)
    o = _np.ones(Bq)
    Hm = _np.stack([p[:, 0], p[:, 1], p[:, 2], p[:, 3], p[:, 4], p[:, 5],
                    z, z, o], -1).reshape(Bq, 3, 3)
    return (warp.astype(_np.float32), conf.astype(_np.float32),
            Hm.astype(_np.float32))


def kernel(feat_A, feat_B, pos_A, pos_B, dustbin_score, geo_weight,
           current_temp, H=64, W=64, **_unused):
    from concourse.bass_utils import run_bass_kernel_spmd

    feat_A = np.asarray(feat_A, np.float32)
    feat_B = np.asarray(feat_B, np.float32)
    pos_A = np.asarray(pos_A, np.float32)
    pos_B = np.asarray(pos_B, np.float32)
    d = float(np.asarray(dustbin_score).reshape(-1)[0])
    gw = float(np.asarray(geo_weight).reshape(-1)[0])
    temp = float(np.asarray(current_temp).reshape(-1)[0])

    scal = np.array([[1.0 / temp, math.exp(d), 1.0 / (2 * N), 0.5, 0.5,
                      temp, 0.0, 0.0]], np.float32)

    nc = _get_nc()
    in_maps = []
    for g in range(8):
        b = g // 4
        r = g % 4
        in_maps.append({
            "fa": np.ascontiguousarray(feat_A[b, r * ROWS:(r + 1) * ROWS]),
            "fb": np.ascontiguousarray(feat_B[b]),
            "scal": scal,
        })
    res = run_bass_kernel_spmd(nc, in_maps, core_ids=list(range(8)))

    vals = np.empty((B, N, TOPK), np.float32)
    idx = np.empty((B, N, TOPK), np.int64)
    for g in range(8):
        b = g // 4
        r = g % 4
        vals[b, r * ROWS:(r + 1) * ROWS] = res.results[g]["vals"]
        idx[b, r * ROWS:(r + 1) * ROWS] = res.results[g]["idx"].astype(np.int64)

    return _host_tail(vals, idx, pos_A, pos_B, gw, temp)


# revision 13
# speedup vs baseline: 1.3082x; 1.3082x over previous
"""AgriMatcher Trainium2 kernel.

Strategy (8 NeuronCores, B=2 batches, N=4096, C=128):
  - Data-parallel over batch: cores 0-3 handle batch 0, cores 4-7 batch 1.
  - Within a batch the [N,N] matrix is row-sharded 4 ways (1024 rows/core).
  - Sinkhorn runs in probability domain: K = exp(S/tau), a = exp(u),
    b = exp(v).  Row pass s = K b is local (b replicated); col pass
    t = K^T a produces partials AllReduce'd over the 4-core group, split
    into two halves so communication overlaps compute.  The dustbin
    row/column are handled algebraically with replicated scalars.
  - K is stored twice in bf16 (row-major for the col pass via PE matvec,
    transposed - produced by DMA-transpose - for the row pass).
  - The final assignment matrix is recomputed in fp32:
    t_ij = exp(cos_ij/tau + ln b_j) via a rank-1 PSUM accumulate, so top-8
    selection and values match the fp32 reference closely.  Top-8 via DVE
    max8 + find_index8.
  - Host does the small tail (geo validation, softmax, affine solve) in
    float64 on [B,N,8] data.
"""

import math
import numpy as np

B = 2
N = 4096
C = 128
P = 128
NT_A = 8     # row tiles per core (1024 rows)
NT_B = 32    # column tiles (full N)
ROWS = NT_A * P
H = W = 64
TOPK = 8
ITERS = 5

_CACHE = {}


def _build():
    import os as _os
    import concourse.bass as bass
    from concourse import bacc, masks
    import concourse.mybir as mybir
    from concourse import tile

    dt = mybir.dt
    AF = mybir.ActivationFunctionType
    DBG = _os.environ.get("AGRI_DEBUG", "0") == "1"

    nc = bacc.Bacc("TRN2", target_bir_lowering=False, debug=False, num_devices=8)

    fa = nc.dram_tensor("fa", [ROWS, C], dt.float32, kind="ExternalInput")
    fb = nc.dram_tensor("fb", [N, C], dt.float32, kind="ExternalInput")
    # scal: [inv_temp, kd, mu, muD, nuD, tau, 0, 0]
    scal = nc.dram_tensor("scal", [1, 8], dt.float32, kind="ExternalInput")
    vals_out = nc.dram_tensor("vals", [ROWS, TOPK], dt.float32, kind="ExternalOutput")
    idx_out = nc.dram_tensor("idx", [ROWS, TOPK], dt.uint32, kind="ExternalOutput")
    if DBG:
        dbg_a5 = nc.dram_tensor("dbg_a5", [P, NT_A], dt.float32, kind="ExternalOutput")
        dbg_b5 = nc.dram_tensor("dbg_b5", [P, NT_B], dt.float32, kind="ExternalOutput")

    with tile.TileContext(nc) as tc:
        with tc.tile_pool(name="persist", bufs=1) as pp, \
             tc.tile_pool(name="dram", bufs=2, space="DRAM") as dram, \
             tc.tile_pool(name="psum", bufs=1, space="PSUM") as psq:
            faT = pp.tile([P, ROWS], dt.float32)         # [c, i] normalized A
            fbT = pp.tile([P, N], dt.float32)            # [c, j] normalized B
            scal_rep = pp.tile([P, 8], dt.float32)
            ones128 = pp.tile([P, 1], dt.float32)
            ones1 = pp.tile([1, P], dt.float32)
            ident = pp.tile([P, P], dt.float32)
            rs_2d = pp.tile([P, NT_A], dt.float32)       # phase-1 row sums
            rs_acc = pp.tile([P, 4], dt.float32)         # per-quarter accums
            a_2d = pp.tile([P, NT_A], dt.float32)
            b_2d = pp.tile([P, NT_B], dt.float32)
            a_bf = pp.tile([P, NT_A], dt.bfloat16)
            b_bf = pp.tile([P, NT_B], dt.bfloat16)
            aD_rep = pp.tile([P, 1], dt.float32)
            bD_rep = pp.tile([P, 1], dt.float32)
            tmp_rep = pp.tile([P, 1], dt.float32)
            tmp_rep2 = pp.tile([P, 1], dt.float32)
            kdbD = pp.tile([P, 1], dt.float32)
            sc1 = pp.tile([1, 1], dt.float32)

            nc.vector.memset(ones128[:], 1.0)
            nc.vector.memset(ones1[:], 1.0)
            masks.make_identity(nc, ident[:])

            INV_T, KD, MU, MUD, NUD, TAU = 0, 1, 2, 3, 4, 5

            def rep_col(k):
                return scal_rep[:, k:k + 1]

            def rep1(dst, src_11):
                """replicate a [1,1] sbuf value to [128,1] via PE rank-1"""
                psr = psq.tile([P, 1], dt.float32, tag="tiny")
                nc.tensor.matmul(psr[:], ones1[:], src_11, start=True, stop=True)
                nc.vector.tensor_copy(dst, psr[:])

            def part_sum(dst_11, src_p1):
                """sum a [128,1] over partitions -> [1,1] sbuf"""
                pss = psq.tile([1, 1], dt.float32, tag="tiny2")
                nc.tensor.matmul(pss[:], ones128[:], src_p1, start=True, stop=True)
                nc.vector.tensor_copy(dst_11, pss[:])

            # ---------------- phase 0: load, normalize, transpose ----------
            with tc.tile_pool(name="ph0", bufs=1) as p0:
                scal_sb = p0.tile([1, 8], dt.float32)
                nc.sync.dma_start(scal_sb[:], scal[:])
                ps_scal = psq.tile([P, 8], dt.float32, tag="tiny")
                nc.tensor.matmul(ps_scal[:], ones1[:], scal_sb[:], start=True, stop=True)
                nc.vector.tensor_copy(scal_rep[:], ps_scal[:])

                fa_buf = p0.tile([P, NT_A, C], dt.float32)
                fb_buf = p0.tile([P, NT_B, C], dt.float32)
                nc.sync.dma_start(fa_buf[:], fa.ap().rearrange("(t p) c -> p t c", p=P))
                nc.sync.dma_start(fb_buf[:], fb.ap().rearrange("(t p) c -> p t c", p=P))

                for name, buf, nt, dst in (("a", fa_buf, NT_A, faT), ("b", fb_buf, NT_B, fbT)):
                    sq = p0.tile([P, nt, C], dt.float32, tag=f"sq{name}")
                    ssq = p0.tile([P, nt], dt.float32, tag=f"ssq{name}")
                    nc.scalar.activation(sq[:], buf[:], AF.Square)
                    nc.vector.tensor_reduce(ssq[:], sq[:], axis=mybir.AxisListType.X,
                                            op=mybir.AluOpType.add)
                    rinv = p0.tile([P, nt], dt.float32, tag=f"rinv{name}")
                    nc.vector.reciprocal(rinv[:], ssq[:])
                    nc.scalar.activation(rinv[:], rinv[:], AF.Sqrt)  # 1/|row|
                    nc.vector.tensor_mul(
                        buf[:], buf[:],
                        rinv[:].unsqueeze(2).broadcast_to([P, nt, C]))
                    # transpose tiles via PE into [c, n] blocks
                    for blk in range((nt + 3) // 4):
                        ntile = min(4, nt - blk * 4)
                        pst = psq.tile([P, 4 * P], dt.float32, tag="q", bufs=2)
                        for q in range(ntile):
                            t_i = blk * 4 + q
                            nc.tensor.transpose(pst[:, q * P:(q + 1) * P],
                                                buf[:, t_i, :], ident[:])
                        nc.vector.tensor_copy(
                            dst[:, blk * 4 * P: blk * 4 * P + ntile * P],
                            pst[:, 0:ntile * P])

            # ---------------- phase 1: S matmul + exp -> K (bf16) ----------
            kp_cm = tc.tile_pool(name="kpool", bufs=1)
            kp = kp_cm.__enter__()
            K_rm = kp.tile([P, NT_A, N], dt.bfloat16)    # row-major K
            KT = kp.tile([P, NT_B, ROWS], dt.bfloat16)   # transposed K
            itv_cm = tc.tile_pool(name="itv", bufs=1)
            itv = itv_cm.__enter__()
            s_row = itv.tile([1, ROWS], dt.float32)
            t_rowh = itv.tile([1, 2048], dt.float32)
            for it in range(NT_A):
                for qq in range(4):
                    ps_s = psq.tile([P, 1024], dt.float32, tag="q", bufs=2)
                    for h in range(2):
                        j0 = qq * 1024 + h * 512
                        nc.tensor.matmul(ps_s[:, h * 512:(h + 1) * 512],
                                         faT[:, it * P:(it + 1) * P],
                                         fbT[:, j0:j0 + 512],
                                         start=True, stop=True)
                    nc.scalar.activation(K_rm[:, it, qq * 1024:(qq + 1) * 1024],
                                         ps_s[:], AF.Exp,
                                         scale=rep_col(INV_T),
                                         accum_out=rs_acc[:, qq:qq + 1])
                nc.vector.tensor_add(rs_2d[:, it:it + 1],
                                     rs_acc[:, 0:1], rs_acc[:, 1:2])
                nc.vector.tensor_add(tmp_rep[:], rs_acc[:, 2:3], rs_acc[:, 3:4])
                nc.vector.tensor_add(rs_2d[:, it:it + 1],
                                     rs_2d[:, it:it + 1], tmp_rep[:])
                # transposed copy on the DMA engine
                nc.sync.dma_start_transpose(
                    KT[:, :, it * P:(it + 1) * P], K_rm[:, it, :])

            # ---------------- Sinkhorn iterations --------------------------
            for itr in range(ITERS):
                # ---- a-update: s_i = sum_j K_ij b_j + kd*bD ----
                if itr == 0:
                    # b = 1: s = rowsums + kd ; aD = muD/(kd*N + 1)
                    nc.vector.tensor_scalar_add(a_2d[:], rs_2d[:], rep_col(KD))
                    nc.vector.reciprocal(a_2d[:], a_2d[:])
                    nc.vector.tensor_scalar_mul(a_2d[:], a_2d[:], rep_col(MU))
                    nc.vector.tensor_scalar_mul(aD_rep[:], ones128[:], rep_col(KD))
                    nc.vector.tensor_scalar(aD_rep[:], aD_rep[:], float(N), 1.0,
                                            op0=mybir.AluOpType.mult,
                                            op1=mybir.AluOpType.add)
                    nc.vector.reciprocal(aD_rep[:], aD_rep[:])
                    nc.vector.tensor_scalar_mul(aD_rep[:], aD_rep[:], rep_col(MUD))
                else:
                    ps_row = psq.tile([1, ROWS], dt.float32, tag="rowvec")
                    for g in range(NT_B):
                        nc.tensor.matmul(ps_row[:, 0:512], b_bf[:, g:g + 1],
                                         KT[:, g, 0:512],
                                         start=(g == 0), stop=False)
                        nc.tensor.matmul(ps_row[:, 512:1024], b_bf[:, g:g + 1],
                                         KT[:, g, 512:1024],
                                         start=(g == 0), stop=(g == NT_B - 1))
                    nc.scalar.copy(s_row[:], ps_row[:])
                    s_dram = dram.tile([ROWS], dt.float32, tag="s_dram")
                    nc.sync.dma_start(s_dram[:], s_row[:])
                    s_2d = a_2d  # reuse storage
                    nc.sync.dma_start(
                        s_2d[:], s_dram[:].rearrange("(t p) -> p t", p=P))
                    nc.vector.tensor_scalar_add(s_2d[:], s_2d[:], kdbD[:])
                    nc.vector.reciprocal(s_2d[:], s_2d[:])
                    nc.vector.tensor_scalar_mul(a_2d[:], s_2d[:], rep_col(MU))
                    # aD = muD / (kd*sum_b + bD)
                    nc.vector.tensor_reduce(tmp_rep[:], b_2d[:],
                                            axis=mybir.AxisListType.X,
                                            op=mybir.AluOpType.add)
                    part_sum(sc1[:], tmp_rep[:])
                    rep1(tmp_rep[:], sc1[:])
                    nc.vector.tensor_mul(tmp_rep[:], tmp_rep[:], rep_col(KD))
                    nc.vector.tensor_add(tmp_rep[:], tmp_rep[:], bD_rep[:])
                    nc.vector.reciprocal(tmp_rep[:], tmp_rep[:])
                    nc.vector.tensor_mul(aD_rep[:], tmp_rep[:], rep_col(MUD))
                if DBG and itr == ITERS - 1:
                    nc.sync.dma_start(dbg_a5.ap(), a_2d[:])
                nc.vector.tensor_copy(a_bf[:], a_2d[:])
                # local sum of a for the dustbin column term
                nc.vector.tensor_reduce(tmp_rep2[:], a_2d[:],
                                        axis=mybir.AxisListType.X,
                                        op=mybir.AluOpType.add)
                part_sum(sc1[:], tmp_rep2[:])

                # ---- col pass: two AR halves, each two psum quarters ----
                ar_halves = []
                for hf in range(2):
                    ar_in = dram.tile([2056], dt.float32, tag=f"ar_in{hf}")
                    ar_out = dram.tile([2056], dt.float32, tag=f"ar_out{hf}")
                    for qq in range(2):
                        ps_col = psq.tile([1, 1024], dt.float32, tag="q", bufs=2)
                        for it in range(NT_A):
                            for h in range(2):
                                j0 = hf * 2048 + qq * 1024 + h * 512
                                nc.tensor.matmul(ps_col[:, h * 512:(h + 1) * 512],
                                                 a_bf[:, it:it + 1],
                                                 K_rm[:, it, j0:j0 + 512],
                                                 start=(it == 0),
                                                 stop=(it == NT_A - 1))
                        nc.scalar.copy(t_rowh[:, qq * 1024:(qq + 1) * 1024], ps_col[:])
                    nc.sync.dma_start(ar_in[0:2048], t_rowh[:, 0:2048])
                    if hf == 0:
                        nc.sync.dma_start(ar_in[2048:2049], sc1[:])
                    nc.gpsimd.collective_compute(
                        "AllReduce", mybir.AluOpType.add,
                        replica_groups=[[0, 1, 2, 3], [4, 5, 6, 7]],
                        ins=[ar_in.opt()], outs=[ar_out.opt()])
                    ar_halves.append(ar_out)

                # ---- b-update (dustbin pieces from half 0's extra slot) ----
                suma_sb = sc1
                nc.sync.dma_start(suma_sb[:], ar_halves[0][2048:2049])
                rep1(tmp_rep[:], suma_sb[:])
                nc.vector.tensor_mul(tmp_rep[:], tmp_rep[:], rep_col(KD))
                nc.vector.tensor_add(tmp_rep[:], tmp_rep[:], aD_rep[:])
                nc.vector.reciprocal(tmp_rep[:], tmp_rep[:])
                nc.vector.tensor_mul(bD_rep[:], tmp_rep[:], rep_col(NUD))
                nc.vector.tensor_mul(kdbD[:], bD_rep[:], rep_col(KD))
                nc.vector.tensor_mul(tmp_rep2[:], aD_rep[:], rep_col(KD))
                for hf in range(2):
                    bh = b_2d[:, hf * 16:(hf + 1) * 16]
                    nc.sync.dma_start(
                        bh, ar_halves[hf][0:2048].rearrange("(g p) -> p g", p=P))
                    nc.vector.tensor_scalar_add(bh, bh, tmp_rep2[:])
                    nc.vector.reciprocal(bh, bh)
                    nc.vector.tensor_scalar_mul(bh, bh, rep_col(MU))
                    nc.vector.tensor_copy(b_bf[:, hf * 16:(hf + 1) * 16], bh)
                if DBG and itr == ITERS - 1:
                    nc.sync.dma_start(dbg_b5.ap(), b_2d[:])

            # ---------------- final: fp32 rescore + top-8 -------------------
            itv_cm.__exit__(None, None, None)
            with tc.tile_pool(name="fin", bufs=2) as fp, \
                 tc.tile_pool(name="fl", bufs=1) as fl, \
                 tc.tile_pool(name="finsmall", bufs=3) as fs:
                lnb_row = fl.tile([1, N], dt.float32)
                lnb_2d = fs.tile([P, NT_B], dt.float32, tag="lnb2d")
                nc.scalar.activation(lnb_2d[:], b_2d[:], AF.Ln)
                nc.vector.tensor_scalar_mul(lnb_2d[:], lnb_2d[:], rep_col(TAU))
                lnb_dram = dram.tile([N], dt.float32, tag="lnb_dram")
                nc.sync.dma_start(
                    lnb_dram[:].rearrange("(g p) -> p g", p=P), lnb_2d[:])
                nc.sync.dma_start(lnb_row[:], lnb_dram[:])

                for it in range(NT_A):
                    tsc = fp.tile([P, N], dt.float32, tag="tscratch")
                    for qq in range(4):
                        ps_t = psq.tile([P, 1024], dt.float32, tag="q", bufs=2)
                        for h in range(2):
                            j0 = qq * 1024 + h * 512
                            nc.tensor.matmul(ps_t[:, h * 512:(h + 1) * 512],
                                             faT[:, it * P:(it + 1) * P],
                                             fbT[:, j0:j0 + 512],
                                             start=True, stop=False)
                        for h in range(2):
                            j0 = qq * 1024 + h * 512
                            nc.tensor.matmul(ps_t[:, h * 512:(h + 1) * 512],
                                             ones1[:],
                                             lnb_row[0:1, j0:j0 + 512],
                                             start=False, stop=True)
                        nc.scalar.activation(tsc[:, qq * 1024:(qq + 1) * 1024],
                                             ps_t[:], AF.Exp, scale=rep_col(INV_T))
                    v8 = fs.tile([P, TOPK], dt.float32, tag="v8")
                    i8 = fs.tile([P, TOPK], dt.uint32, tag="i8")
                    nc.vector.max(v8[:], tsc[:])
                    nc.vector.max_index(i8[:], v8[:], tsc[:])
                    nc.vector.tensor_scalar_mul(v8[:], v8[:], a_2d[:, it:it + 1])
                    nc.sync.dma_start(
                        vals_out.ap().rearrange("(t p) k -> p t k", p=P)[:, it, :],
                        v8[:])
                    nc.sync.dma_start(
                        idx_out.ap().rearrange("(t p) k -> p t k", p=P)[:, it, :],
                        i8[:])

            kp_cm.__exit__(None, None, None)

    nc.compile()
    return nc


def _get_nc():
    if "nc" not in _CACHE:
        _CACHE["nc"] = _build()
    return _CACHE["nc"]


def _host_tail(vals, idx, pos_A, pos_B, gw, temp):
    """vals/idx: [B, N, 8];  tail in float64 on host."""
    import numpy as _np
    Bq, Nq, K = vals.shape
    vals = vals.astype(_np.float64)
    pa = pos_A.astype(_np.float64)
    pb = pos_B.astype(_np.float64)
    gw = float(_np.clip(gw, 0.0, 2.0))
    bidx = _np.arange(Bq)[:, None, None]
    tpos = pb[bidx, idx]                      # [B,N,K,2]
    disp = (tpos - pa[:, :, None, :]).reshape(Bq, H, W, K, 2)

    def box(x):
        # 7x7 zero-padded box sum via cumsum
        for ax in (1, 2):
            cs = _np.cumsum(x, axis=ax)
            n = x.shape[ax]
            hi = _np.clip(_np.arange(n) + 3, 0, n - 1)
            lo = _np.arange(n) - 4
            hi_s = _np.take(cs, hi, axis=ax)
            lo_s = _np.take(cs, _np.clip(lo, 0, n - 1), axis=ax)
            mask_shape = (1,) * ax + (n,) + (1,) * (x.ndim - ax - 1)
            lo_s = _np.where((lo >= 0).reshape(mask_shape), lo_s, 0.0)
            x = hi_s - lo_s
        return x

    cnt = box(_np.ones((1, H, W, 1, 1)))
    mean = box(disp) / cnt
    mean_sq = box(disp * disp) / cnt
    var = _np.clip(mean_sq - mean * mean, 0.0, None)
    geo = 1.0 / (1.0 + 100.0 * var.sum(-1).reshape(Bq, Nq, K))

    comb = vals + gw * geo
    e = _np.exp((comb - comb.max(-1, keepdims=True)) / temp)
    soft = e / e.sum(-1, keepdims=True)
    warp = (tpos * soft[..., None]).sum(2)
    conf = (soft * vals).sum(-1, keepdims=True)

    w = conf[..., 0]
    w = w / _np.clip(w.sum(1, keepdims=True), 1e-6, None)
    x, y = pa[..., 0], pa[..., 1]
    one = _np.ones_like(x)
    zero = _np.zeros_like(x)
    rx = _np.stack([x, y, one, zero, zero, zero], -1)
    ry = _np.stack([zero, zero, zero, x, y, one], -1)
    A = _np.concatenate([rx, ry], 1)
    bb = _np.concatenate([warp[..., 0], warp[..., 1]], 1)[..., None]
    wd = _np.concatenate([w, w], 1)[..., None]
    Aw = A * wd
    AtWA = _np.einsum('bni,bnj->bij', Aw, A) + 1e-4 * _np.eye(6)
    AtWb = _np.einsum('bni,bnk->bik', Aw, bb)
    p = _np.linalg.solve(AtWA, AtWb)[..., 0]
    z = _np.zeros(Bq)
    o = _np.ones(Bq)
    Hm = _np.stack([p[:, 0], p[:, 1], p[:, 2], p[:, 3], p[:, 4], p[:, 5],
                    z, z, o], -1).reshape(Bq, 3, 3)
    return (warp.astype(_np.float32), conf.astype(_np.float32),
            Hm.astype(_np.float32))


def kernel(feat_A, feat_B, pos_A, pos_B, dustbin_score, geo_weight,
           current_temp, H=64, W=64, **_unused):
    from concourse.bass_utils import run_bass_kernel_spmd

    feat_A = np.asarray(feat_A, np.float32)
    feat_B = np.asarray(feat_B, np.float32)
    pos_A = np.asarray(pos_A, np.float32)
    pos_B = np.asarray(pos_B, np.float32)
    d = float(np.asarray(dustbin_score).reshape(-1)[0])
    gw = float(np.asarray(geo_weight).reshape(-1)[0])
    temp = float(np.asarray(current_temp).reshape(-1)[0])

    scal = np.array([[1.0 / temp, math.exp(d), 1.0 / (2 * N), 0.5, 0.5,
                      temp, 0.0, 0.0]], np.float32)

    nc = _get_nc()
    in_maps = []
    for g in range(8):
        b = g // 4
        r = g % 4
        in_maps.append({
            "fa": np.ascontiguousarray(feat_A[b, r * ROWS:(r + 1) * ROWS]),
            "fb": np.ascontiguousarray(feat_B[b]),
            "scal": scal,
        })
    res = run_bass_kernel_spmd(nc, in_maps, core_ids=list(range(8)))

    vals = np.empty((B, N, TOPK), np.float32)
    idx = np.empty((B, N, TOPK), np.int64)
    for g in range(8):
        b = g // 4
        r = g % 4
        vals[b, r * ROWS:(r + 1) * ROWS] = res.results[g]["vals"]
        idx[b, r * ROWS:(r + 1) * ROWS] = res.results[g]["idx"].astype(np.int64)

    return _host_tail(vals, idx, pos_A, pos_B, gw, temp)
